# revision 1
# baseline (speedup 1.0000x reference)
"""Multi-head cross-attention Trainium2 kernel (8 NeuronCores, SPMD).

Problem: nn_MultiHeadCrossAttention_31791347925263
  x:[4,2048,768], y:[4,2048,768], 12 heads x 64, fp32.
  out = softmax((x Wq^T)(y Wk^T)^T / 8 + mask) (y Wv^T) Wo^T   (+ zero biases)

Sharding: 8 cores = (batch b in 0..3) x (query half in 0..1). Each core
computes the full attention for its 1024 query rows against all 2048 keys
of its batch. No collectives; outputs concatenate.

Per-core dataflow (all matmuls in float32r = TF32-like, 11-bit mantissa RNE):
  host:  xT=[768,1024], yT=[768,2048], WqT/WkT/WvT/WoT = W.T contiguous
         (k/v rows of Wkv are interleaved per head: 64 k then 64 v per 128)
  kT  = WkT-blocks^T-matmul yT      -> [768(k-dim), 2048(sk)]
  v'  = yT-blocks^T-matmul WvT      -> [2048(sk), 780] (65 cols/head: v|ones)
  qT  = WqT-blocks^T-matmul xT      -> [768(q-dim), 1024(sq)]
  per head pair (2*hb, 2*hb+1), per sk-block (128 keys):
      S^T = kT_h-block^T-matmul qT_h      -> PSUM [128, 1024] (row groups
                                             0-63/64-127 alternate -> the two
                                             heads' K=64 matmuls overlap)
      P~  = exp(S^T * 0.125)              -> SBUF f32r (ACT, no max-subtract:
                                             scores ~ N(0,1), max ~ 4)
      valT_h += v'[skb,h]^T-matmul P~     -> PSUM [65, 1024]
                                             (row 64 = softmax denominator)
  valnorm_h = valT_h[0:64] * bcast(1/valT_h[64])
      (DVE copy + fast-reciprocal, GPSIMD partition-broadcast; valnorm
       overwrites qT's tiles - same [128,1024] f32r shape, qT[hb] is dead
       once pair hb's QK matmuls are done)
  o[sqb]    = valnorm-blocks^T-matmul WoT -> [1024, 768] -> DMA out

All matmul outputs except valT share one 2-buf [128,1024] PSUM tag
(2 banks per slot; valT pool 2x2 banks) = exactly the 8 PSUM banks.
"""

import numpy as np

B, S, D = 4, 2048, 768
H, Dh = 12, 64
SQ = S // 2          # queries per core
N_CORES = 8
DB = D // 128        # 6 d_model blocks
SKB = S // 128       # 16 key blocks
SQB = SQ // 128      # 8 query blocks per core
VPW = H * (Dh + 1)   # 780: v' width (64 v cols + 1 ones col per head)

_cache = {}


def _build_nc():
    import concourse.mybir as mybir
    import concourse.tile as tile
    from concourse import bacc

    f32 = mybir.dt.float32
    f32r = mybir.dt.float32r
    EXP = mybir.ActivationFunctionType.Exp

    nc = bacc.Bacc("TRN2", target_bir_lowering=False)
    xT = nc.dram_tensor("xT", [D, SQ], f32, kind="ExternalInput")
    yT = nc.dram_tensor("yT", [D, S], f32, kind="ExternalInput")
    WqT = nc.dram_tensor("WqT", [D, D], f32, kind="ExternalInput")
    WkT = nc.dram_tensor("WkT", [D, D], f32, kind="ExternalInput")
    WvT = nc.dram_tensor("WvT", [D, D], f32, kind="ExternalInput")
    WoT = nc.dram_tensor("WoT", [D, D], f32, kind="ExternalInput")
    out = nc.dram_tensor("out", [SQ, D], f32, kind="ExternalOutput")

    with tile.TileContext(nc) as tc:
        with tc.tile_pool(name="persist", bufs=1) as pp, \
             tc.tile_pool(name="mmps", bufs=2, space="PSUM") as mm_ps, \
             tc.tile_pool(name="vtp", bufs=2, space="PSUM") as vt_ps:

            def mm_tile(cols):
                return mm_ps.tile([128, cols], f32, name="mmps", tag="mmps",
                                  padded_shape=[128, SQ])

            kT = [pp.tile([128, S], f32r, name=f"kT{i}") for i in range(DB)]
            vp = [pp.tile([128, VPW], f32r, name=f"vp{i}") for i in range(SKB)]
            qT = [pp.tile([128, SQ], f32r, name=f"qT{i}") for i in range(DB)]
            vnorm = qT  # valnorm overwrites qT (same shape; see docstring)

            with tc.tile_pool(name="ld_y", bufs=1) as ld_y:
                yTs = [ld_y.tile([128, S], f32r, name=f"yTs{i}")
                       for i in range(DB)]

                # ---- kT projection: kT[ob] = (WkT col-block)^T @ yT ----
                with tc.tile_pool(name="ld_wk", bufs=1) as ld_wk:
                    wkTs = [ld_wk.tile([128, D], f32r, name=f"wkTs{i}")
                            for i in range(DB)]
                    for i in range(DB):
                        nc.sync.dma_start(
                            out=wkTs[i],
                            in_=WkT[i * 128:(i + 1) * 128, :].bitcast(f32r))
                    for c4 in range(4):
                        for i in range(DB):
                            nc.sync.dma_start(
                                out=yTs[i][:, c4 * 512:(c4 + 1) * 512],
                                in_=yT[i * 128:(i + 1) * 128,
                                       c4 * 512:(c4 + 1) * 512].bitcast(f32r))
                    wvTs = [ld_y.tile([128, D], f32r, name=f"wvTs{i}")
                            for i in range(DB)]
                    for i in range(DB):
                        nc.sync.dma_start(
                            out=wvTs[i],
                            in_=WvT[i * 128:(i + 1) * 128, :].bitcast(f32r))
                    # nc4 outer: the first 6 groups need only yT column
                    # chunk 0, so compute starts while chunks 1-3 stream in
                    for nc4 in range(4):
                        for ob in range(DB):
                            ps = mm_tile(512)
                            for kb in range(DB):
                                nc.tensor.matmul(
                                    ps[:, :],
                                    wkTs[kb][:, ob * 128:(ob + 1) * 128],
                                    yTs[kb][:, nc4 * 512:(nc4 + 1) * 512],
                                    start=(kb == 0), stop=(kb == DB - 1))
                            nc.vector.tensor_copy(
                                kT[ob][:, nc4 * 512:(nc4 + 1) * 512], ps[:, :])

                # ---- v' projection: v[skb] = (yT blk)^T @ WvT ----
                if True:
                    for skb in range(SKB):
                        vps3 = vp[skb].rearrange("p (h c) -> p h c", c=Dh + 1)
                        nc.vector.memset(vps3[:, :, Dh].bitcast(f32), 1.0)
                        for nc2 in range(2):
                            n0, n1 = nc2 * 512, min(D, (nc2 + 1) * 512)
                            ps = mm_tile(512)
                            for kb in range(DB):
                                nc.tensor.matmul(
                                    ps[:, 0:n1 - n0],
                                    yTs[kb][:, skb * 128:(skb + 1) * 128],
                                    wvTs[kb][:, n0:n1],
                                    start=(kb == 0), stop=(kb == DB - 1))
                            # contiguous v-cols -> 65-strided layout
                            src = ps[:, 0:n1 - n0].rearrange(
                                "p (h c) -> p h c", c=Dh)
                            dst = vps3[:, nc2 * 8:nc2 * 8 + (n1 - n0) // Dh,
                                       0:Dh]
                            nc.vector.tensor_copy(dst, src)

            # ---- qT projection ----
            with tc.tile_pool(name="ld_x", bufs=1) as ld_x:
                xTs = [ld_x.tile([128, SQ], f32r, name=f"xTs{i}")
                       for i in range(DB)]
                wqTs = [ld_x.tile([128, D], f32r, name=f"wqTs{i}")
                        for i in range(DB)]
                # critical-path order: weights, then xT halves in chunk order
                for i in range(DB):
                    nc.sync.dma_start(
                        out=wqTs[i],
                        in_=WqT[i * 128:(i + 1) * 128, :].bitcast(f32r))
                for c2 in range(2):
                    for i in range(DB):
                        nc.sync.dma_start(
                            out=xTs[i][:, c2 * 512:(c2 + 1) * 512],
                            in_=xT[i * 128:(i + 1) * 128,
                                   c2 * 512:(c2 + 1) * 512].bitcast(f32r))
                for nc2 in range(2):
                    for ob in range(DB):
                        ps = mm_tile(512)
                        for kb in range(DB):
                            nc.tensor.matmul(
                                ps[:, :],
                                wqTs[kb][:, ob * 128:(ob + 1) * 128],
                                xTs[kb][:, nc2 * 512:(nc2 + 1) * 512],
                                start=(kb == 0), stop=(kb == DB - 1))
                        nc.vector.tensor_copy(
                            qT[ob][:, nc2 * 512:(nc2 + 1) * 512], ps[:, :])

            # ---- attention ----
            with tc.tile_pool(name="late", bufs=1) as lp:
                woT = [lp.tile([128, D], f32r, name=f"woT{i}")
                       for i in range(DB)]
                for i in range(DB):
                    nc.sync.dma_start(
                        out=woT[i],
                        in_=WoT[i * 128:(i + 1) * 128, :].bitcast(f32r))

                with tc.tile_pool(name="psb", bufs=5) as p_pool, \
                     tc.tile_pool(name="nrm", bufs=2) as nrm_pool:
                    for hb in range(H // 2):
                        h0, h1 = 2 * hb, 2 * hb + 1
                        vt0 = vt_ps.tile([65, SQ], f32, name="valT")
                        vt1 = vt_ps.tile([65, SQ], f32, name="valT")
                        for skb in range(SKB):
                            st0 = mm_tile(SQ)
                            st1 = mm_tile(SQ)
                            for j in range(2):
                                for r0, st in ((0, st0), (64, st1)):
                                    nc.tensor.matmul(
                                        st[:, j * 512:(j + 1) * 512],
                                        kT[hb][r0:r0 + 64,
                                               skb * 128:(skb + 1) * 128],
                                        qT[hb][r0:r0 + 64,
                                               j * 512:(j + 1) * 512],
                                        start=True, stop=True)
                            pt0 = p_pool.tile([128, SQ], f32r, name="pT")
                            pt1 = p_pool.tile([128, SQ], f32r, name="pT")
                            nc.scalar.activation(pt0[:, :], st0[:, :], EXP,
                                                 scale=0.125)
                            nc.scalar.activation(pt1[:, :], st1[:, :], EXP,
                                                 scale=0.125)
                            for h, vt, pt in ((h0, vt0, pt0), (h1, vt1, pt1)):
                                for j in range(2):
                                    nc.tensor.matmul(
                                        vt[:, j * 512:(j + 1) * 512],
                                        vp[skb][:, h * 65:h * 65 + 65],
                                        pt[:, j * 512:(j + 1) * 512],
                                        start=(skb == 0),
                                        stop=(skb == SKB - 1))
                        for h, vt in ((h0, vt0), (h1, vt1)):
                            r0 = (h % 2) * 64
                            # single fast copy frees the PSUM accumulator so
                            # the next pair's PV can start immediately
                            vals = nrm_pool.tile([65, SQ], f32, name="vals")
                            nc.vector.tensor_copy(vals[:, :], vt[:, :])
                            rec = nrm_pool.tile([1, SQ], f32, name="rec")
                            nc.vector.reciprocal(rec[:, :], vals[64:65, :])
                            rbc = nrm_pool.tile([64, SQ], f32, name="rbc")
                            nc.gpsimd.partition_broadcast(rbc[:, :], rec[:, :])
                            nc.vector.tensor_mul(
                                vnorm[hb][r0:r0 + 64, :], vals[0:64, :],
                                rbc[:, :])

                # ---- output projection ----
                # alternate PSUM slots between the mm pool and the (now idle)
                # valT pool -> 4 concurrent accumulation groups instead of 2
                with tc.tile_pool(name="osb", bufs=3) as o_pool:
                    for sqb in range(SQB):
                        if sqb % 2 == 0:
                            op = mm_tile(D)
                        else:
                            op = vt_ps.tile([128, D], f32, name="valT",
                                            tag="valT",
                                            padded_shape=[128, SQ])
                        for nc2 in range(2):
                            n0, n1 = nc2 * 512, min(D, (nc2 + 1) * 512)
                            for kb in range(DB):
                                nc.tensor.matmul(
                                    op[:, n0:n1],
                                    vnorm[kb][:, sqb * 128:(sqb + 1) * 128],
                                    woT[kb][:, n0:n1],
                                    start=(kb == 0), stop=(kb == DB - 1))
                        ot = o_pool.tile([128, D], f32, name="osb")
                        nc.vector.tensor_copy(ot[:, :], op[:, :])
                        nc.sync.dma_start(
                            out=out[sqb * 128:(sqb + 1) * 128, :], in_=ot[:, :])

    nc.compile()
    return nc


def _get_nc():
    if "nc" not in _cache:
        _cache["nc"] = _build_nc()
    return _cache["nc"]


def _host_fallback(x, y, mask, Wq, bq, Wkv, bkv, Wo, bo):
    Bb, Ss, _ = x.shape
    q = x @ Wq.T + bq
    kv = y @ Wkv.T + bkv
    q = q.reshape(Bb, Ss, H, Dh).transpose(0, 2, 1, 3)
    kv = kv.reshape(Bb, Ss, H, 2 * Dh).transpose(0, 2, 1, 3)
    k, v = kv[..., :Dh], kv[..., Dh:]
    scaled = np.einsum("bhqd,bhkd->bhqk", q, k) / np.sqrt(np.float32(Dh))
    scaled = scaled + mask
    scaled -= scaled.max(axis=-1, keepdims=True)
    e = np.exp(scaled)
    attn = e / e.sum(axis=-1, keepdims=True)
    values = np.einsum("bhqk,bhkd->bhqd", attn, v)
    values = values.transpose(0, 2, 1, 3).reshape(Bb, Ss, H * Dh)
    return (values @ Wo.T + bo).astype(np.float32)


def _run(inputs, trace=False, trace_cores=None):
    """Returns (full_output, BassKernelResults)."""
    from concourse.bass_utils import run_bass_kernel_spmd

    x = np.ascontiguousarray(np.asarray(inputs["x"], dtype=np.float32))
    y = np.ascontiguousarray(np.asarray(inputs["y"], dtype=np.float32))
    Wq = np.asarray(inputs["Wq"], dtype=np.float32)
    Wkv = np.asarray(inputs["Wkv"], dtype=np.float32)
    Wo = np.asarray(inputs["Wo"], dtype=np.float32)

    # Reference reshapes kv to [B,S,H,2*Dh]: per head, rows h*128..h*128+63 of
    # Wkv are the k-projection, rows h*128+64..h*128+127 the v-projection.
    k_rows = np.concatenate([np.arange(h * 128, h * 128 + Dh) for h in range(H)])
    v_rows = np.concatenate([np.arange(h * 128 + Dh, (h + 1) * 128)
                             for h in range(H)])
    WqT = np.ascontiguousarray(Wq.T)
    WkT = np.ascontiguousarray(Wkv[k_rows].T)
    WvT = np.ascontiguousarray(Wkv[v_rows].T)
    WoT = np.ascontiguousarray(Wo.T)

    in_maps = []
    for c in range(N_CORES):
        b, half = c // 2, c % 2
        xTc = np.ascontiguousarray(x[b, half * SQ:(half + 1) * SQ, :].T)
        yTb = np.ascontiguousarray(y[b].T)
        in_maps.append({"xT": xTc, "yT": yTb, "WqT": WqT, "WkT": WkT,
                        "WvT": WvT, "WoT": WoT})

    nc = _get_nc()
    res = run_bass_kernel_spmd(nc, in_maps, core_ids=list(range(N_CORES)),
                               trace=trace, trace_cores=trace_cores)
    out = np.empty((B, S, D), dtype=np.float32)
    for c in range(N_CORES):
        b, half = c // 2, c % 2
        out[b, half * SQ:(half + 1) * SQ, :] = res.results[c]["out"]
    return out, res


def kernel(**inputs) -> np.ndarray:
    mask = np.asarray(inputs["mask"], dtype=np.float32)
    bq = np.asarray(inputs["bq"], dtype=np.float32)
    bkv = np.asarray(inputs["bkv"], dtype=np.float32)
    bo = np.asarray(inputs["bo"], dtype=np.float32)
    if mask.any() or bq.any() or bkv.any() or bo.any():
        # Device kernel hardcodes zero mask/biases; stay correct regardless.
        return _host_fallback(
            np.asarray(inputs["x"], dtype=np.float32),
            np.asarray(inputs["y"], dtype=np.float32),
            mask, np.asarray(inputs["Wq"], dtype=np.float32), bq,
            np.asarray(inputs["Wkv"], dtype=np.float32), bkv,
            np.asarray(inputs["Wo"], dtype=np.float32), bo)
    out, _ = _run(inputs)
    return out



# revision 9
# speedup vs baseline: 1.1034x; 1.1034x over previous
"""Multi-head cross-attention Trainium2 kernel (8 NeuronCores, SPMD).

Problem: nn_MultiHeadCrossAttention_31791347925263
  x:[4,2048,768], y:[4,2048,768], 12 heads x 64, fp32.
  out = softmax((x Wq^T)(y Wk^T)^T / 8 + mask) (y Wv^T) Wo^T   (+ zero biases)

Sharding: 8 cores = (batch b in 0..3) x (query half in 0..1). Each core
computes the full attention for its 1024 query rows against all 2048 keys
of its batch. No collectives; outputs concatenate.

v2 design (from trace analysis of v1 at 415us):
  * v1's attention phase was ACT(exp)-bound: 25.2M exps/core at 153G/s
    = 220us on the scalar engine alone. v2 splits exp 50/50:
      - ACT: exact exp of the j0 query-half scores ([128,1024] per skb).
      - DVE: one fused custom-DVE instruction (registered at import:
        EXP_POLY4_ANT = sq(sq(1 + u(a + u(b + u c))))) evaluating
        exp(0.125*u) to ~3e-3 weighted rel err for the j1 half.
        Softmax normalization cancels any per-query scale, and the fit
        is softmax-weighted (exact where e^s is large); end-to-end adds
        ~1.3e-3 rel err (sim; gate is 2e-2).
  * QK pairs issued adjacently so the two K=64 matmuls run concurrently
    in disjoint PE row groups (v1's schedule separated them).
  * Score tiles stj0/stj1 [128,1024] hold BOTH heads side by side
    (cols 0-511 head h0, 512-1023 head h1), so one ACT/DVE instruction
    covers the pair. PSUM: stj0(2)+stj1(2)+vt0(2)+vt1(2) = 8 banks.
  * PV lags one skb behind QK so the PE never waits on exp.
  * v1's 12x 6.5us single-partition RECIPROCAL -> reciprocal_approx_fast
    + gpsimd broadcast + gpsimd multiply (off the critical engines).
  * Batched 3D DMAs (one dispatch per tensor/chunk instead of 6).

Per-core dataflow (matmuls fp32r, moving dim 512, contraction via PSUM):
  kT  = WkT-blocks^T-matmul yT      -> [768(k-dim), 2048(sk)]
  v'  = yT-blocks^T-matmul WvT      -> [2048(sk), 780] (65 cols/head: v|ones)
  qT  = WqT-blocks^T-matmul xT      -> [768(q-dim), 1024(sq)]
  attention per head pair / skb as above; row 64 of vt = softmax denom.
  vnorm overwrites qT; o = vnorm-blocks^T-matmul WoT -> [1024,768] -> DMA.
"""

import numpy as np

B, S, D = 4, 2048, 768
H, Dh = 12, 64
SQ = S // 2          # queries per core
N_CORES = 8
DB = D // 128        # 6 d_model blocks
SKB = S // 128       # 16 key blocks
SQB = SQ // 128      # 8 query blocks per core
VPW = H * (Dh + 1)   # 780: v' width (64 v cols + 1 ones col per head)

# exp(0.125*u) ~= p(u)^4, p = 1 + u(a + u(b + u c)); coeffs are the
# softmax-weighted minimax fit of e^v on v=0.125*u/4 folded with the scales.
_EA, _EB, _EC = 0.99989984, 0.46879885, 0.24913378
EXP_C0 = _EA / 32.0
EXP_C1 = _EB / 1024.0
EXP_C2 = _EC / 32768.0

_cache = {}


def _register_exp_op():
    """Register the fused exp custom-DVE op (documented extension point of
    concourse.dve_ops; the repo is read-only in this image so the op is
    appended to the in-process tables instead of the source file)."""
    if "exp_op" in _cache:
        return _cache["exp_op"]
    import concourse.dve_ops as dvo
    from concourse.dve_ops import DveOp
    from concourse.dve_spec import C0, C1, C2, One, Spec, Src0, lower, sq
    from concourse.dve_uop import DveOpSpec

    def _ref(in0, in1, s0, s1, imm2):
        x = in0.astype(np.float32)
        p = (1.0 + x * (s0 + x * (s1 + x * imm2))).astype(np.float32)
        p2 = (p * p).astype(np.float32)
        return (p2 * p2).astype(np.float32)

    spec = Spec(
        body=sq(sq(One + Src0 * (C0 + Src0 * (C1 + Src0 * C2)))),
        reference=_ref,
    )
    name = "EXP_POLY4_ANT"
    if name not in dvo._SUB_OPCODE_FOR_NAME:
        row = max(dvo._SUB_OPCODE_FOR_NAME.values()) + 1
        assert row < 0x20
        dvo._SUB_OPCODE_FOR_NAME[name] = row
    shas = {}
    for ver in ("v3",):
        uops = lower(spec, ver=ver)
        shas[ver] = DveOpSpec(
            name=name, opcode=dvo._SUB_OPCODE_FOR_NAME[name], uops=uops,
            rd1_en=False,
        ).sha(ver)
    op = DveOp(name, spec, subdim=False, uops_sha=shas)
    if not any(o.name == name for o in dvo.OPS):
        dvo.OPS.append(op)
        dvo.CUSTOM_DVE_SPECS[name] = spec
    _cache["exp_op"] = op
    return op


def _build_nc():
    import concourse.mybir as mybir
    import concourse.tile as tile
    from concourse import bacc

    f32 = mybir.dt.float32
    f32r = mybir.dt.float32r
    EXP = mybir.ActivationFunctionType.Exp
    exp_op = _register_exp_op()

    nc = bacc.Bacc("TRN2", target_bir_lowering=False)
    xT = nc.dram_tensor("xT", [D, SQ], f32, kind="ExternalInput")
    yT = nc.dram_tensor("yT", [D, S], f32, kind="ExternalInput")
    WqT = nc.dram_tensor("WqT", [D, D], f32, kind="ExternalInput")
    WkT = nc.dram_tensor("WkT", [D, D], f32, kind="ExternalInput")
    WvT = nc.dram_tensor("WvT", [D, D], f32, kind="ExternalInput")
    WoT = nc.dram_tensor("WoT", [D, D], f32, kind="ExternalInput")
    out = nc.dram_tensor("out", [SQ, D], f32, kind="ExternalOutput")

    def custom_exp(out_ap, in_ap):
        nc.vector._custom_dve(
            exp_op, out=out_ap, in0=in_ap,
            s0=EXP_C0, s1=EXP_C1, imm2=EXP_C2)

    with tile.TileContext(nc) as tc:
        with tc.tile_pool(name="persist", bufs=1) as pp:
            kT = [pp.tile([128, S], f32r, name=f"kT{i}") for i in range(DB)]
            vp = [pp.tile([128, VPW], f32r, name=f"vp{i}") for i in range(SKB)]
            qT = [pp.tile([128, SQ], f32r, name=f"qT{i}") for i in range(DB)]
            vnorm = qT  # valnorm overwrites qT (same shape; see docstring)

            # preload the exp activation table while DMAs run
            warm = pp.tile([1, 16], f32, name="warm")
            nc.vector.memset(warm[:, :], 0.0)
            nc.scalar.activation(warm[:, :], warm[:, :], EXP)

            proj_ctx = tc.tile_pool(name="projps", bufs=2, space="PSUM")
            proj_ps = proj_ctx.__enter__()

            def proj_tile(cols):
                return proj_ps.tile([128, cols], f32, name="projps",
                                    tag="projps", padded_shape=[128, 512])

            with tc.tile_pool(name="ld_y", bufs=1) as ld_y:
                yTs = ld_y.tile([128, DB * S], f32r, name="yTs")
                yT3 = yTs.rearrange("p (i c) -> p i c", i=DB)

                # ---- kT projection: kT[ob] = (WkT col-block)^T @ yT ----
                with tc.tile_pool(name="ld_wk", bufs=1) as ld_wk:
                    wkTs = ld_wk.tile([128, DB * D], f32r, name="wkTs")
                    wk3 = wkTs.rearrange("p (i c) -> p i c", i=DB)
                    for i in range(DB):
                        nc.sync.dma_start(
                            out=wk3[:, i, :],
                            in_=WkT[i * 128:(i + 1) * 128, :].bitcast(f32r))
                    for c4 in range(4):
                        for i in range(DB):
                            nc.sync.dma_start(
                                out=yT3[:, i, c4 * 512:(c4 + 1) * 512],
                                in_=yT[i * 128:(i + 1) * 128,
                                       c4 * 512:(c4 + 1) * 512].bitcast(f32r))
                    wvTs = ld_y.tile([128, DB * D], f32r, name="wvTs")
                    wv3 = wvTs.rearrange("p (i c) -> p i c", i=DB)
                    for i in range(DB):
                        nc.sync.dma_start(
                            out=wv3[:, i, :],
                            in_=WvT[i * 128:(i + 1) * 128, :].bitcast(f32r))
                    # nc4 outer: the first 6 groups need only yT column
                    # chunk 0, so compute starts while chunks 1-3 stream in
                    for nc4 in range(4):
                        for ob in range(DB):
                            ps = proj_tile(512)
                            for kb in range(DB):
                                nc.tensor.matmul(
                                    ps[:, :],
                                    wk3[:, kb, ob * 128:(ob + 1) * 128],
                                    yT3[:, kb, nc4 * 512:(nc4 + 1) * 512],
                                    start=(kb == 0), stop=(kb == DB - 1))
                            dst = kT[ob][:, nc4 * 512:(nc4 + 1) * 512]
                            if ob % 2 == 0:
                                nc.vector.tensor_copy(dst, ps[:, :])
                            else:
                                nc.scalar.copy(dst, ps[:, :])

                # ---- v' projection: v[skb] = (yT blk)^T @ WvT ----
                for skb in range(SKB):
                    vps3 = vp[skb].rearrange("p (h c) -> p h c", c=Dh + 1)
                    nc.vector.memset(vps3[:, :, Dh].bitcast(f32), 1.0)
                    for nc2 in range(2):
                        n0, n1 = nc2 * 512, min(D, (nc2 + 1) * 512)
                        ps = proj_tile(512)
                        for kb in range(DB):
                            nc.tensor.matmul(
                                ps[:, 0:n1 - n0],
                                yT3[:, kb, skb * 128:(skb + 1) * 128],
                                wv3[:, kb, n0:n1],
                                start=(kb == 0), stop=(kb == DB - 1))
                        # contiguous v-cols -> 65-strided layout
                        src = ps[:, 0:n1 - n0].rearrange(
                            "p (h c) -> p h c", c=Dh)
                        dst = vps3[:, nc2 * 8:nc2 * 8 + (n1 - n0) // Dh, 0:Dh]
                        if skb % 2 == 0:
                            nc.vector.tensor_copy(dst, src)
                        else:
                            nc.scalar.copy(dst, src)

            # ---- qT projection ----
            with tc.tile_pool(name="ld_x", bufs=1) as ld_x:
                xTs = ld_x.tile([128, DB * SQ], f32r, name="xTs")
                xT3 = xTs.rearrange("p (i c) -> p i c", i=DB)
                wqTs = ld_x.tile([128, DB * D], f32r, name="wqTs")
                wq3 = wqTs.rearrange("p (i c) -> p i c", i=DB)
                for i in range(DB):
                    nc.sync.dma_start(
                        out=wq3[:, i, :],
                        in_=WqT[i * 128:(i + 1) * 128, :].bitcast(f32r))
                for c2 in range(2):
                    for i in range(DB):
                        nc.sync.dma_start(
                            out=xT3[:, i, c2 * 512:(c2 + 1) * 512],
                            in_=xT[i * 128:(i + 1) * 128,
                                   c2 * 512:(c2 + 1) * 512].bitcast(f32r))
                for nc2 in range(2):
                    for ob in range(DB):
                        ps = proj_tile(512)
                        for kb in range(DB):
                            nc.tensor.matmul(
                                ps[:, :],
                                wq3[:, kb, ob * 128:(ob + 1) * 128],
                                xT3[:, kb, nc2 * 512:(nc2 + 1) * 512],
                                start=(kb == 0), stop=(kb == DB - 1))
                        dst = qT[ob][:, nc2 * 512:(nc2 + 1) * 512]
                        if ob % 2 == 0:
                            nc.vector.tensor_copy(dst, ps[:, :])
                        else:
                            nc.scalar.copy(dst, ps[:, :])

            proj_ctx.__exit__(None, None, None)

            # ---- attention ----
            with tc.tile_pool(name="late", bufs=1) as lp:
                woT = lp.tile([128, DB * D], f32r, name="woT")
                wo3 = woT.rearrange("p (i c) -> p i c", i=DB)
                for i in range(DB):
                    nc.sync.dma_start(
                        out=wo3[:, i, :],
                        in_=WoT[i * 128:(i + 1) * 128, :].bitcast(f32r))

                with tc.tile_pool(name="stp0", bufs=1, space="PSUM") as stp0, \
                     tc.tile_pool(name="stp1", bufs=1, space="PSUM") as stp1, \
                     tc.tile_pool(name="vtp", bufs=2, space="PSUM") as vtp, \
                     tc.tile_pool(name="ptp", bufs=2) as ptp, \
                     tc.tile_pool(name="nrm", bufs=2) as nrm_pool:
                    for hb in range(H // 2):
                        h0, h1 = 2 * hb, 2 * hb + 1
                        vt0 = vtp.tile([65, SQ], f32, name="valT", tag="valT")
                        vt1 = vtp.tile([65, SQ], f32, name="valT", tag="valT")

                        def emit_pv(skb, pt0, pt1, vt0=vt0, vt1=vt1,
                                    h0=h0, h1=h1):
                            st = (skb == 0)
                            sp = (skb == SKB - 1)
                            for vt, h, hc in ((vt0, h0, 0), (vt1, h1, 512)):
                                nc.tensor.matmul(
                                    vt[:, 0:512],
                                    vp[skb][:, h * 65:h * 65 + 65],
                                    pt0[:, hc:hc + 512], start=st, stop=sp)
                            for vt, h, hc in ((vt0, h0, 0), (vt1, h1, 512)):
                                nc.tensor.matmul(
                                    vt[:, 512:1024],
                                    vp[skb][:, h * 65:h * 65 + 65],
                                    pt1[:, hc:hc + 512], start=st, stop=sp)

                        prev = None
                        for skb in range(SKB):
                            s0t = stp0.tile([128, SQ], f32, name="stj0",
                                            tag="stj0")
                            s1t = stp1.tile([128, SQ], f32, name="stj1",
                                            tag="stj1")
                            sk = slice(skb * 128, (skb + 1) * 128)
                            # j0 pair then j1 pair; the two K=64 matmuls of
                            # each pair sit in disjoint row groups and run
                            # concurrently
                            nc.tensor.matmul(
                                s0t[:, 0:512], kT[hb][0:64, sk],
                                qT[hb][0:64, 0:512], start=True, stop=True)
                            nc.tensor.matmul(
                                s0t[:, 512:1024], kT[hb][64:128, sk],
                                qT[hb][64:128, 0:512], start=True, stop=True)
                            nc.tensor.matmul(
                                s1t[:, 0:512], kT[hb][0:64, sk],
                                qT[hb][0:64, 512:1024], start=True, stop=True)
                            nc.tensor.matmul(
                                s1t[:, 512:1024], kT[hb][64:128, sk],
                                qT[hb][64:128, 512:1024],
                                start=True, stop=True)
                            # PV of the previous skb goes into the PE queue
                            # before this skb's exp so the PE never waits
                            if prev is not None:
                                emit_pv(*prev)
                            pt0 = ptp.tile([128, SQ], f32r, name="pt0",
                                           tag="pt0")
                            pt1 = ptp.tile([128, SQ], f32r, name="pt1",
                                           tag="pt1")
                            nc.scalar.activation(pt0[:, :], s0t[:, :], EXP,
                                                 scale=0.125)
                            custom_exp(pt1[:, :], s1t[:, :])
                            prev = (skb, pt0, pt1)
                        emit_pv(*prev)

                        for h, vt in ((h0, vt0), (h1, vt1)):
                            r0 = (h % 2) * 64
                            vals = nrm_pool.tile([65, SQ], f32, name="vals")
                            # split the PSUM drain across both engines
                            nc.scalar.copy(vals[:, 0:512], vt[:, 0:512])
                            nc.vector.tensor_copy(vals[:, 512:1024],
                                                  vt[:, 512:1024])
                            # engines can't read 1 partition at base 64 or
                            # broadcast from it; hop the denominator row to
                            # partition 0 with a tiny SBUF->SBUF DMA first
                            dnm = nrm_pool.tile([1, SQ], f32, name="dnm")
                            nc.sync.dma_start(out=dnm[:, :],
                                              in_=vals[64:65, :])
                            rbc = nrm_pool.tile([64, SQ], f32, name="rbc")
                            nc.gpsimd.partition_broadcast(rbc[:, :],
                                                          dnm[:, :])
                            rec = nrm_pool.tile([64, SQ], f32, name="rec")
                            nc.vector.reciprocal_approx_fast(
                                out=rec[:, :], in_=rbc[:, :])
                            nc.vector.tensor_mul(
                                vnorm[hb][r0:r0 + 64, :],
                                vals[0:64, :], rec[:, :])

                # ---- output projection ----
                with tc.tile_pool(name="ops", bufs=4, space="PSUM") as o_ps, \
                     tc.tile_pool(name="osb", bufs=3) as o_pool:
                    for sqb in range(SQB):
                        op = o_ps.tile([128, D], f32, name="ops", tag="ops")
                        for nc2 in range(2):
                            n0, n1 = nc2 * 512, min(D, (nc2 + 1) * 512)
                            for kb in range(DB):
                                nc.tensor.matmul(
                                    op[:, n0:n1],
                                    vnorm[kb][:, sqb * 128:(sqb + 1) * 128],
                                    wo3[:, kb, n0:n1],
                                    start=(kb == 0), stop=(kb == DB - 1))
                        ot = o_pool.tile([128, D], f32, name="osb")
                        nc.scalar.copy(ot[:, :], op[:, :])
                        nc.sync.dma_start(
                            out=out[sqb * 128:(sqb + 1) * 128, :],
                            in_=ot[:, :])

    nc.compile()
    return nc


def _get_nc():
    if "nc" not in _cache:
        _cache["nc"] = _build_nc()
    return _cache["nc"]


def _host_fallback(x, y, mask, Wq, bq, Wkv, bkv, Wo, bo):
    Bb, Ss, _ = x.shape
    q = x @ Wq.T + bq
    kv = y @ Wkv.T + bkv
    q = q.reshape(Bb, Ss, H, Dh).transpose(0, 2, 1, 3)
    kv = kv.reshape(Bb, Ss, H, 2 * Dh).transpose(0, 2, 1, 3)
    k, v = kv[..., :Dh], kv[..., Dh:]
    scaled = np.einsum("bhqd,bhkd->bhqk", q, k) / np.sqrt(np.float32(Dh))
    scaled = scaled + mask
    scaled -= scaled.max(axis=-1, keepdims=True)
    e = np.exp(scaled)
    attn = e / e.sum(axis=-1, keepdims=True)
    values = np.einsum("bhqk,bhkd->bhqd", attn, v)
    values = values.transpose(0, 2, 1, 3).reshape(Bb, Ss, H * Dh)
    return (values @ Wo.T + bo).astype(np.float32)


def _run(inputs, trace=False, trace_cores=None):
    """Returns (full_output, BassKernelResults)."""
    from concourse.bass_utils import run_bass_kernel_spmd

    x = np.ascontiguousarray(np.asarray(inputs["x"], dtype=np.float32))
    y = np.ascontiguousarray(np.asarray(inputs["y"], dtype=np.float32))
    Wq = np.asarray(inputs["Wq"], dtype=np.float32)
    Wkv = np.asarray(inputs["Wkv"], dtype=np.float32)
    Wo = np.asarray(inputs["Wo"], dtype=np.float32)

    # Reference reshapes kv to [B,S,H,2*Dh]: per head, rows h*128..h*128+63 of
    # Wkv are the k-projection, rows h*128+64..h*128+127 the v-projection.
    k_rows = np.concatenate([np.arange(h * 128, h * 128 + Dh)
                             for h in range(H)])
    v_rows = np.concatenate([np.arange(h * 128 + Dh, (h + 1) * 128)
                             for h in range(H)])
    WqT = np.ascontiguousarray(Wq.T)
    WkT = np.ascontiguousarray(Wkv[k_rows].T)
    WvT = np.ascontiguousarray(Wkv[v_rows].T)
    WoT = np.ascontiguousarray(Wo.T)

    in_maps = []
    for c in range(N_CORES):
        b, half = c // 2, c % 2
        xTc = np.ascontiguousarray(x[b, half * SQ:(half + 1) * SQ, :].T)
        yTb = np.ascontiguousarray(y[b].T)
        in_maps.append({"xT": xTc, "yT": yTb, "WqT": WqT, "WkT": WkT,
                        "WvT": WvT, "WoT": WoT})

    nc = _get_nc()
    res = run_bass_kernel_spmd(nc, in_maps, core_ids=list(range(N_CORES)),
                               trace=trace, trace_cores=trace_cores)
    out = np.empty((B, S, D), dtype=np.float32)
    for c in range(N_CORES):
        b, half = c // 2, c % 2
        out[b, half * SQ:(half + 1) * SQ, :] = res.results[c]["out"]
    return out, res


def kernel(**inputs) -> np.ndarray:
    mask = np.asarray(inputs["mask"], dtype=np.float32)
    bq = np.asarray(inputs["bq"], dtype=np.float32)
    bkv = np.asarray(inputs["bkv"], dtype=np.float32)
    bo = np.asarray(inputs["bo"], dtype=np.float32)
    if mask.any() or bq.any() or bkv.any() or bo.any():
        # Device kernel hardcodes zero mask/biases; stay correct regardless.
        return _host_fallback(
            np.asarray(inputs["x"], dtype=np.float32),
            np.asarray(inputs["y"], dtype=np.float32),
            mask, np.asarray(inputs["Wq"], dtype=np.float32), bq,
            np.asarray(inputs["Wkv"], dtype=np.float32), bkv,
            np.asarray(inputs["Wo"], dtype=np.float32), bo)
    out, _ = _run(inputs)
    return out
